# revision 7
# baseline (speedup 1.0000x reference)
"""Trainium2 Bass kernel for stereo disparity correlation (cost volume).

corr[b, d, h, w] = (1/C) * sum_c L[b,c,h,w] * R[b,c,h,w-d],  zero-padded w-d<0
x = concat([L, R], axis=1): [4, 64, 256, 512] f32, C=32, D=64.

Sharding: 8 cores = 4 batches x 2 H-halves. Each core handles
x_shard [64, 128, 512] -> out_shard [64(D), 128(H), 512(W)].

Algorithm per core (memory-bound; ~64MB HBM traffic/core):
  1. Load L,R rows (4 h at a time, [128p, 512] f32), cast to fp16 on DVE
     (L scaled by 1/C; R into a 63-col zero-padded tile).
  2. Per (h, wblock k of 128): PE Gram G[r, n] = sum_c Ls[c, 128k+r] *
     Rpad[c, 128k+n], K=32, M=128, N=191.  out[w=128k+r, d] = G[r, r+63-d].
  3. ACT-evict G (f32 PSUM -> fp16 SBUF), DMA to a flat DRAM scratch tile.
  4. Re-read the diagonal band via flat AP: off = r*769 + k*192 + d',
     d' = 63-d  -> band[r, k, d'] in SBUF.
  5. PE transpose band k-slices -> PSUM [d', w], DVE-evict f32.
  6. DMA to output with negative outer step on d' (flips to d).
"""

import sys
from contextlib import ExitStack

import numpy as np

for _p in ("/opt/trn_rl_repo",):
    if _p not in sys.path:
        sys.path.insert(0, _p)

import concourse.bass as bass
import concourse.bacc as bacc
import concourse.tile as tile
from concourse import mybir
from concourse.bass_utils import run_bass_kernel_spmd

# problem constants (hardcoded per contract)
B, C2, H, W = 4, 64, 256, 512
C = 32
D = 64
N_CORES = 8
HS = H // 2          # h rows per core = 128
KB = W // 128        # w-blocks per row = 4
NCOL = 191           # Gram columns per block (n = r + d', r<=127, d'<=63)
NPAD = 191           # scratch row (no padding; band flat stride = KB*NPAD+1)
GH = 4               # h rows loaded per input DMA group

FP32 = mybir.dt.float32
FP16 = mybir.dt.float16


def build_kernel(hs=HS):
    nc = bacc.Bacc(
        "TRN2", target_bir_lowering=False, debug=False, num_devices=N_CORES
    )
    x = nc.dram_tensor("x", [C2, hs, W], FP32, kind="ExternalInput").ap()
    ident = nc.dram_tensor("ident", [128, 128], FP16, kind="ExternalInput").ap()
    out = nc.dram_tensor("out", [D, hs, W], FP32, kind="ExternalOutput").ap()

    with tile.TileContext(nc) as tc, ExitStack() as ctx:
        lio = ctx.enter_context(tc.tile_pool(name="lio", bufs=3))
        lcast = ctx.enter_context(tc.tile_pool(name="lcast", bufs=3))
        gbuf = ctx.enter_context(tc.tile_pool(name="gbuf", bufs=3))
        bbuf = ctx.enter_context(tc.tile_pool(name="bbuf", bufs=3))
        obuf = ctx.enter_context(tc.tile_pool(name="obuf", bufs=3))
        singles = ctx.enter_context(tc.tile_pool(name="singles", bufs=1))
        psg = ctx.enter_context(tc.tile_pool(name="psg", bufs=4, space="PSUM"))
        pso = ctx.enter_context(tc.tile_pool(name="pso", bufs=2, space="PSUM"))
        dram = ctx.enter_context(tc.tile_pool(name="dram", bufs=4, space="DRAM"))

        ident_t = singles.tile([128, 128], FP16)
        nc.sync.dma_start(out=ident_t[:], in_=ident)

        n_groups = hs // GH
        for g in range(n_groups):
            h0 = g * GH
            # ---- load 4 h rows of L and R, f32 ----
            lf32 = lio.tile([128, W], FP32, tag="lf32")
            rf32 = lio.tile([128, W], FP32, tag="rf32")
            # dest partitions (h, c): one DMA per h row (partition dim must
            # be a single leading AP dim on the SBUF side)
            for hi in range(GH):
                nc.sync.dma_start(
                    out=lf32[C * hi : C * hi + C, :], in_=x[0:C, h0 + hi, :]
                )
                nc.sync.dma_start(
                    out=rf32[C * hi : C * hi + C, :],
                    in_=x[C : 2 * C, h0 + hi, :],
                )
            # ---- cast to fp16 (L scaled by 1/C), R zero-padded by 63 ----
            ls = lcast.tile([128, W], FP16, tag="ls")
            rpad = lcast.tile([128, 63 + W], FP16, tag="rpad")
            nc.vector.tensor_scalar_mul(ls[:], lf32[:], 1.0 / C)
            nc.vector.memset(rpad[:], 0.0)
            nc.vector.tensor_copy(rpad[:, 63 : 63 + W], rf32[:])

            for hpair in range(GH // 2):
                # two h rows share one psum_o / out tile / out DMA
                pso_t = pso.tile([128, W], FP16, tag="pso")
                band2 = [None, None]
                for hi2 in range(2):
                    hi = hpair * 2 + hi2
                    h = h0 + hi
                    prow = slice(C * hi, C * hi + C)
                    gt = gbuf.tile([128, KB, NPAD], FP16, tag="gt")
                    for k in range(KB):
                        psg_t = psg.tile([128, NCOL], FP32, tag="psg")
                        nc.tensor.matmul(
                            psg_t[:],
                            ls[prow, 128 * k : 128 * k + 128],
                            rpad[prow, 128 * k : 128 * k + NCOL],
                            start=True,
                            stop=True,
                            tile_position=(C * hi, 0),
                        )
                        nc.scalar.copy(gt[:, k, 0:NCOL], psg_t[:])
                    # scratch roundtrip: write G, read back diagonal band
                    gd = dram.tile([128, KB, NPAD], FP16, tag="gd")
                    nc.sync.dma_start(out=gd[:], in_=gt[:])
                    band = bbuf.tile([128, KB, D], FP16, tag="band")
                    gd_ap = gd[:]
                    band_src = bass.AP(
                        tensor=gd_ap.tensor,
                        offset=gd_ap.offset,
                        ap=[[NPAD * KB + 1, 128], [NPAD, KB], [1, D]],
                    )
                    nc.sync.dma_start(out=band[:], in_=band_src)
                    band2[hi2] = band
                    for k in range(KB):
                        nc.tensor.transpose(
                            pso_t[64 * hi2 : 64 * hi2 + 64, 128 * k : 128 * k + 128],
                            band[:, k, :],
                            ident_t[:],
                        )
                # evict both rows, write out with d flip
                out_t = obuf.tile([128, W], FP32, tag="out_t")
                nc.vector.tensor_copy(out_t[:], pso_t[:])
                ho = h0 + hpair * 2
                dst = bass.AP(
                    tensor=out.tensor,
                    offset=(D - 1) * hs * W + ho * W,
                    ap=[[W, 2], [-hs * W, D], [1, W]],
                )
                nc.sync.dma_start(out=dst, in_=out_t[:])

    nc.compile()
    return nc


_NC_CACHE = {}


def _get_nc(hs=HS):
    if hs not in _NC_CACHE:
        _NC_CACHE[hs] = build_kernel(hs)
    return _NC_CACHE[hs]


def make_in_maps(x_full):
    ident = np.eye(128, dtype=np.float16)
    in_maps = []
    for core in range(N_CORES):
        b, hh = core // 2, core % 2
        shard = np.ascontiguousarray(x_full[b, :, hh * HS : (hh + 1) * HS, :])
        in_maps.append({"x": shard, "ident": ident})
    return in_maps


def assemble(results):
    out = np.empty((B, D, H, W), dtype=np.float32)
    for core in range(N_CORES):
        b, hh = core // 2, core % 2
        out[b, :, hh * HS : (hh + 1) * HS, :] = results[core]["out"]
    return out


def kernel(x, max_disp):
    x = np.asarray(x, dtype=np.float32)
    assert x.shape == (B, C2, H, W) and int(max_disp) == D
    nc = _get_nc()
    res = run_bass_kernel_spmd(nc, make_in_maps(x), list(range(N_CORES)))
    return assemble(res.results)
